# revision 44
# baseline (speedup 1.0000x reference)
"""DTW teacher-feature expansion kernel for Trainium2 (8 NeuronCores, data parallel).

For each of 16 (teacher[400,1024], student[600,1024]) pairs:
  D = pairwise euclidean distance, R = DTW accumulated-cost DP, exact
  backtrack path (argmin over diag/up/left, diag preferred on ties),
  expanded[j] += teacher[i] over path cells. Returns [16,600,1024] f32.

Per core (2 samples):
  1. D = sqrt(relu(aa + bb - 2 t@s^T)) via PE matmuls.
  2. Forward DP row-by-row; one fused tensor_tensor_scan per row:
     R[j] = min(min(U[j],U[j-1]), R[j-1]) + D[j]
     (op0=min over data0=min(U,U<<1), op1=add over data1=D).
  3. Bulk PACK table from R (grid layout, all rows parallel): choice masks
     L/Dn, then ONE running-max scan of val = 2*slot - Dn - BIG*L gives, for
     every (row i, entry col j), s = 2*g - d at the exit cell g (last
     non-left cell <= j) with diag flag d.  PACK[i,j] = lo*1024 + nextj
     where lo = s - s>>1 (exit col) and nextj = s>>1 (entry col of row i-1).
  4. Backtrack = 400-step scalar pointer chase (one dynamic reg_load per
     row from partition-0-staged PACK chunks; sample 0 on the SP engine,
     sample 1 on the Activation engine, running concurrently).
  5. W rows rebuilt in bulk from per-row lo/hi scalars; out = W^T @ teacher
     via PE matmuls in bf16 (W is 0/1 exactly; teacher rounded to bf16,
     ~2^-9 relative error, far under the 2e-2 gate).

The chase reproduces the reference backtrack exactly given R; R is bit-exact
given D (min exact; one fp32 add per cell). D differs from the grader's only
in matmul reduction order (~1e-6 abs), far below path-decision margins (~1e-2).
"""
import os
import sys

for _p in ("/opt/trn_rl_repo", "/root/.axon_site/_ro/trn_rl_repo"):
    if os.path.isdir(_p) and _p not in sys.path:
        sys.path.insert(0, _p)

import numpy as np
from contextlib import ExitStack

import concourse.bass as bass
import concourse.bacc as bacc
import concourse.mybir as mybir
from concourse import tile

F32 = mybir.dt.float32
BF16 = mybir.dt.bfloat16
I32 = mybir.dt.int32
AOT = mybir.AluOpType
ACTF = mybir.ActivationFunctionType

B, T1, T2, DM = 16, 400, 600, 1024
NCORES, SPC = 8, 2
BIG = 1.0e30
BIGI = 1.0e9                 # index-domain BIG for the running-max scan
KCH = DM // 128              # 8 K-chunks for the D matmul
ICH = (T1 + 127) // 128      # 4 i-chunks: 128,128,128,16
NH = 2                       # n-halves of 300 for the D matmul
MT = 120                     # M-tile of the output matmul (600 = 5*120)
NBLK = ICH * SPC             # 8 (c,s) blocks in the grids
GW2 = NBLK * 601             # grid width, pitch 601: pad + 600 cells
GCH = 8                      # chase chunk = 8 rows
STG_BUFS = 2                 # staging buffers (prefetch depth 1)
NCHUNK = T1 // GCH           # 50 chase chunks per sample


def _ich(c):
    return min(128, T1 - 128 * c)


def blk(c, s):
    return (c * SPC + s) * 601 + 1


def build_kernel(nc, tT=None, sT=None, tnat=None, out=None, dbg=None):
    if tT is None:
        tT = nc.dram_tensor("tT", [SPC, DM, T1], F32, kind="ExternalInput")
        sT = nc.dram_tensor("sT", [SPC, DM, T2], F32, kind="ExternalInput")
        tnat = nc.dram_tensor("tnat", [SPC, T1, DM], BF16, kind="ExternalInput")
        out = nc.dram_tensor("out", [SPC, T2, DM], F32, kind="ExternalOutput")

    with ExitStack() as ctx, tile.TileContext(nc) as tc:
        esD, esR, esP = ExitStack(), ExitStack(), ExitStack()
        with tc.tile_pool(name="pWgp", bufs=1) as pWgp, \
             tc.tile_pool(name="pcst", bufs=1) as pcst:
            # jrow[p, j] = j (f32)
            jrow = pcst.tile([128, T2], F32, tag="jrow")
            nc.gpsimd.iota(jrow[:, :], pattern=[[1, T2]], base=0,
                           channel_multiplier=0,
                           allow_small_or_imprecise_dtypes=True)
            pPk = esP.enter_context(tc.tile_pool(name="pPk", bufs=1))
            gpool = esD.enter_context(tc.tile_pool(name="pDp", bufs=1))
            # D rows, i-partitioned: [i%128, (i//128 *2 + s)*600 + j]
            Dg = gpool.tile([128, ICH * SPC * T2], F32, tag="Dg")
            nc.vector.memset(Dg[:, :], 0.0)

            # ---------------- phase 1: D ----------------
            # Both samples' prep is done first, then the D chunks for both
            # samples interleaved c ascending, so phase 2 (which consumes
            # [2,600]-batched rows) can start after chunk 0 instead of after
            # the whole of sample 1.
            with tc.tile_pool(name="ph1", bufs=1) as p1, \
                 tc.tile_pool(name="pp1", bufs=1, space="PSUM") as pp1:
                ones = p1.tile([128, 1], F32, tag="ones")
                nc.vector.memset(ones[:, :], 1.0)
                tTa_, sTa_, bb_bc_, aa_ = [], [], [], []
                for s in range(SPC):
                    stg1 = p1.tile([128, KCH * T2], F32, tag="stg1", name="stg1")
                    tTa = p1.tile([128, KCH * T1], F32, tag=f"tTa{s}",
                                  name=f"tTa{s}")
                    sTa = p1.tile([128, KCH * T2], F32, tag=f"sTa{s}",
                                  name=f"sTa{s}")
                    nc.sync.dma_start(
                        stg1[:, 0:KCH * T1],
                        tT[s, :, :].rearrange("(k p) i -> p k i", p=128))
                    # PE may carry only one sem wait; launder DMA deps via DVE
                    nc.vector.tensor_copy(out=tTa[:, :], in_=stg1[:, 0:KCH * T1])
                    nc.sync.dma_start(
                        stg1[:, :], sT[s, :, :].rearrange("(k p) j -> p k j", p=128))
                    nc.vector.tensor_copy(out=sTa[:, :], in_=stg1[:, :])
                    tTt = [tTa[:, k * T1:(k + 1) * T1] for k in range(KCH)]
                    sTt = [sTa[:, k * T2:(k + 1) * T2] for k in range(KCH)]
                    # bb[j] = sum_k s[j,k]^2 via ones-matmul over squared sT
                    ps_bb = [pp1.tile([1, 300], F32, tag=f"psbb{h}", name=f"psbb{h}")
                             for h in range(NH)]
                    for k in range(KCH):
                        sq = p1.tile([128, T2], F32, tag="sq", name="sq", bufs=2)
                        nc.vector.tensor_tensor(
                            out=sq[:, :], in0=sTt[k], in1=sTt[k],
                            op=AOT.mult)
                        for h in range(NH):
                            nc.tensor.matmul(ps_bb[h][:, :], lhsT=ones[:, :],
                                             rhs=sq[:, 300 * h:300 * (h + 1)],
                                             start=(k == 0), stop=(k == KCH - 1))
                    bb_sb = p1.tile([1, T2], F32, tag="bbsb")
                    for h in range(NH):
                        nc.vector.tensor_copy(out=bb_sb[:, 300 * h:300 * (h + 1)],
                                              in_=ps_bb[h][:, :])
                    bb_bc = p1.tile([128, T2], F32, tag=f"bbbc{s}",
                                    name=f"bbbc{s}")
                    nc.gpsimd.partition_broadcast(bb_bc[:, :], bb_sb[:, :])
                    # aa[i] via ones-matmul over squared tTa
                    ps_aa = pp1.tile([1, T1], F32, tag="psaa")
                    for k in range(KCH):
                        sqt = p1.tile([128, T1], F32, tag="sqt", name="sqt", bufs=2)
                        nc.vector.tensor_tensor(
                            out=sqt[:, :], in0=tTt[k], in1=tTt[k], op=AOT.mult)
                        nc.tensor.matmul(ps_aa[:, :], lhsT=ones[:, :],
                                         rhs=sqt[:, :],
                                         start=(k == 0), stop=(k == KCH - 1))
                    aa_sb = p1.tile([1, T1], F32, tag="aasb")
                    nc.vector.tensor_copy(out=aa_sb[:, :], in_=ps_aa[:, :])
                    aa = [p1.tile([128, 1], F32, tag=f"aa{s}_{c}",
                                  name=f"aa{s}_{c}")
                          for c in range(ICH)]
                    for c in range(ICH):
                        h = _ich(c)
                        nc.sync.dma_start(aa[c][0:h, 0:1],
                                          aa_sb[0:1, 128 * c:128 * c + h])
                    tTa_.append(tTt)
                    sTa_.append(sTt)
                    bb_bc_.append(bb_bc)
                    aa_.append(aa)
                # ab then D = sqrt(relu(-2ab + bb + aa)), chunk-interleaved
                for c in range(ICH):
                    h = _ich(c)
                    for s in range(SPC):
                        tTt, sTt = tTa_[s], sTa_[s]
                        bb_bc, aa = bb_bc_[s], aa_[s]
                        for n2 in range(NH):
                            ps_ab = pp1.tile([128, 300], F32, tag="psab")
                            for k in range(KCH):
                                nc.tensor.matmul(
                                    ps_ab[0:h, :],
                                    lhsT=tTt[k][:, 128 * c:128 * c + h],
                                    rhs=sTt[k][:, 300 * n2:300 * (n2 + 1)],
                                    start=(k == 0), stop=(k == KCH - 1))
                            u = p1.tile([128, 300], F32, tag="u")
                            nc.vector.scalar_tensor_tensor(
                                out=u[0:h, :], in0=ps_ab[0:h, :], scalar=-2.0,
                                in1=bb_bc[0:h, 300 * n2:300 * (n2 + 1)],
                                op0=AOT.mult, op1=AOT.add)
                            nc.vector.tensor_scalar(
                                out=u[0:h, :], in0=u[0:h, :],
                                scalar1=aa[c][0:h, 0:1], scalar2=0.0,
                                op0=AOT.add, op1=AOT.max)
                            db = (c * SPC + s) * T2 + 300 * n2
                            nc.scalar.activation(
                                out=Dg[0:h, db:db + 300], in_=u[0:h, :],
                                func=ACTF.Sqrt)

            # ------- phase 2+3: forward DP fused with per-chunk PACK --------
            # Rr / RrU: i-partitioned row grids, pitch 601 per (c,s) block:
            # slot(i, j, s) = blk(c,s) + j at partition i%128.
            # RrU holds R[i-1, *] at row i's slot (vertical-shift duplicate),
            # built per chunk by a partition-shifted DMA as soon as the DP
            # passes the chunk.  PACK blocks are then computed ON THE POOL
            # ENGINE (DVE keeps running the DP rows), so phase 3 costs almost
            # nothing on the critical path and the backtrack chase can start
            # right after the last DP row.
            RS = 4
            NG = T1 // RS
            pRp = esR.enter_context(tc.tile_pool(name="pRp", bufs=1))
            Rr = pRp.tile([128, GW2], F32, tag="Rr")
            RrU = pRp.tile([128, GW2], F32, tag="RrU")
            nc.vector.memset(Rr[:, :], BIG)
            nc.vector.memset(RrU[:, :], BIG)
            PACKi = pPk.tile([128, GW2], I32, tag="PACKi")

            def p3_block(p3m, c, s):
                # per-block PACK build, local slot indices k=0..600 (cells at
                # k=1+j): val = 2k - BIGI*L - Dn, running max s, lo_l=s-s>>1,
                # nj_l = s>>1, PACK = lo_l*1024 + nj_l - 1025.
                # Masks via min/eq:  L = cl < min(cd,cu);  Dn = cd==min3.
                # Pool (HW-legal subset: iota/copy/sub/add+max) takes what it
                # can; comparisons, mins, scan, shift, stt stay on DVE.
                o0 = (c * SPC + s) * 601
                cl = Rr[:, o0:o0 + 600]
                cu = RrU[:, o0 + 1:o0 + 601]
                cd = RrU[:, o0:o0 + 600]
                vbb = p3m.tile([128, 601], F32, tag="vbb", name="vbb")
                nc.gpsimd.iota(vbb[:, :], pattern=[[2, 601]], base=0,
                               channel_multiplier=0,
                               allow_small_or_imprecise_dtypes=True)
                m2 = p3m.tile([128, 600], F32, tag="m2", name="m2")
                wa = p3m.tile([128, 600], F32, tag="wa", name="wa")
                wb = p3m.tile([128, 600], F32, tag="wb", name="wb")
                nc.vector.tensor_tensor(out=m2[:, :], in0=cd, in1=cu,
                                        op=AOT.min)
                nc.vector.tensor_tensor(out=wa[:, :], in0=cl, in1=m2[:, :],
                                        op=AOT.is_lt)        # wa := L
                nc.vector.tensor_tensor(out=m2[:, :], in0=m2[:, :], in1=cl,
                                        op=AOT.min)          # m2 := min3
                nc.vector.tensor_tensor(out=wb[:, :], in0=cd, in1=m2[:, :],
                                        op=AOT.is_equal)     # wb := Dn
                cells = vbb[:, 1:601]
                nc.vector.scalar_tensor_tensor(
                    out=cells, in0=wa[:, :], scalar=-BIGI,
                    in1=cells, op0=AOT.mult, op1=AOT.add)
                nc.gpsimd.tensor_tensor(
                    out=cells, in0=cells, in1=wb[:, :], op=AOT.subtract)
                sc = p3m.tile([128, 601], F32, tag="sc", name="sc")
                nc.vector.tensor_tensor_scan(
                    out=sc[:, :], data0=vbb[:, :], data1=vbb[:, :],
                    initial=-2.0 * BIGI, op0=AOT.max, op1=AOT.max)
                si = p3m.tile([128, 601], I32, tag="si", name="si")
                fi = p3m.tile([128, 601], I32, tag="fi", name="fi")
                nc.gpsimd.tensor_copy(out=si[:, :], in_=sc[:, :])
                nc.vector.tensor_scalar(out=fi[:, :], in0=si[:, :],
                                        scalar1=1, scalar2=0,
                                        op0=AOT.arith_shift_right,
                                        op1=AOT.bitwise_or)
                nc.gpsimd.tensor_tensor(out=si[:, :], in0=si[:, :],
                                        in1=fi[:, :], op=AOT.subtract)
                nc.vector.scalar_tensor_tensor(
                    out=fi[:, :], in0=si[:, :], scalar=1024,
                    in1=fi[:, :], op0=AOT.mult, op1=AOT.add)
                nc.gpsimd.tensor_scalar(
                    out=PACKi[:, o0:o0 + 601], in0=fi[:, :],
                    scalar1=-1025, scalar2=0, op0=AOT.add, op1=AOT.max)

            with tc.tile_pool(name="p3m", bufs=1) as p3m, \
                 tc.tile_pool(name="ph2", bufs=1) as p2, \
                 tc.tile_pool(name="pdr", bufs=3) as pdr, \
                 tc.tile_pool(name="prr", bufs=3) as prr:
                bigrow = p2.tile([SPC, T2], F32, tag="bigrow")
                nc.vector.memset(bigrow[:, :], BIG)
                # pre-set the per-row BIG pad column in all rotating rs bufs
                for _ in range(3):
                    rs0 = prr.tile([SPC, RS * (T2 + 1)], F32, tag="rs", name="rs")
                    nc.vector.memset(
                        rs0[:, :].rearrange("a (r c2) -> a r c2", r=RS)[:, :, 0:1],
                        BIG)
                prev = None
                for g in range(NG):
                    i0 = g * RS
                    c, p0 = i0 // 128, i0 % 128
                    ds_ = pdr.tile([SPC, RS * T2], F32, tag="ds", name="ds")
                    for s in range(SPC):
                        nc.sync.dma_start(
                            ds_[s:s + 1, :],
                            Dg[p0:p0 + RS, (c * SPC + s) * T2:(c * SPC + s + 1) * T2])
                    rs_ = prr.tile([SPC, RS * (T2 + 1)], F32, tag="rs", name="rs")
                    for r in range(RS):
                        i = i0 + r
                        rb = r * (T2 + 1)
                        dr = ds_[:, r * T2:(r + 1) * T2]
                        rr = rs_[:, rb:rb + T2 + 1]
                        if i == 0:
                            nc.vector.tensor_tensor_scan(
                                out=rr[:, 1:T2 + 1], data0=bigrow[:, :], data1=dr,
                                initial=0.0, op0=AOT.min, op1=AOT.add)
                        else:
                            m = p2.tile([SPC, T2], F32, tag="m", bufs=2)
                            nc.vector.tensor_tensor(
                                out=m[:, :], in0=prev[:, 1:T2 + 1],
                                in1=prev[:, 0:T2], op=AOT.min)
                            nc.vector.tensor_tensor_scan(
                                out=rr[:, 1:T2 + 1], data0=m[:, :], data1=dr,
                                initial=BIG, op0=AOT.min, op1=AOT.add)
                        prev = rr
                    # contiguous row-major stores: rows -> Rr (Act DMA queue)
                    for s in range(SPC):
                        rv = rs_[s:s + 1, :].rearrange(
                            "a (r c2) -> a r c2", r=RS)[:, :, 1:T2 + 1]
                        nc.scalar.dma_start(
                            Rr[p0:p0 + RS, blk(c, s):blk(c, s) + T2].opt(),
                            rv.opt())
                    iend = i0 + RS - 1
                    if iend % 128 == 127 or iend == T1 - 1:
                        # chunk c complete: build its RrU rows (partition-
                        # shifted DMA, Act queue - off the DVE stream)
                        hh = _ich(c)
                        nc.scalar.dma_start(
                            RrU[1:hh, c * SPC * 601:(c + 1) * SPC * 601],
                            Rr[0:hh - 1, c * SPC * 601:(c + 1) * SPC * 601])
                        if c > 0:
                            for s in range(SPC):
                                nc.scalar.dma_start(
                                    RrU[0:1, blk(c, s):blk(c, s) + T2],
                                    Rr[127:128, blk(c - 1, s):blk(c - 1, s) + T2])
                        for s in range(SPC):
                            p3_block(p3m, c, s)

            if dbg is not None:
                nc.sync.dma_start(dbg["Dg"][:, :], Dg[:, :])
                nc.sync.dma_start(dbg["Rg0"][:, :], Rr[:, :])
                nc.sync.dma_start(dbg["PACK"][:, :], PACKi[:, :])
            # Free Rr/RrU: all later pools allocate on the RIGHT side of
            # SBUF, so this space is never re-aliased (aliasing would add
            # write-after-read waits on all phase-3 reads).
            esR.close()

            # ---------------- phase 4: scalar pointer-chase backtrack -------
            # vout[s][c][p] = PACK[128c+p, j_e]  (lo*1024 + nextj).  hi of a
            # row is nextj of the row above it, so each per-chunk tile gets
            # one extra slot holding the first row of the chunk above (or the
            # 599 sentinel at the very top).  Per-chunk tiles (not one big
            # tile) so downstream DMAs fire as soon as the chase passes a
            # chunk, letting W-build + matmuls overlap the chase.
            CH_ENG = ("sync", "scalar")
            vout = [[pWgp.tile([1, _ich(c) + 1], I32, tag=f"vo{s}_{c}",
                               name=f"vo{s}_{c}")
                     for c in range(ICH)] for s in range(SPC)]
            for s in range(SPC):
                h3 = _ich(ICH - 1)
                nc.vector.memset(vout[s][ICH - 1][0:1, h3:h3 + 1], T2 - 1)
            # prefetch teacher (rhs of phase 5) on the Pool DMA queue
            tn = []
            for s in range(SPC):
                tnr = pWgp.tile([128, ICH * DM], BF16, tag=f"tn{s}",
                                name=f"tn{s}")
                for c in range(ICH):
                    h = _ich(c)
                    nc.gpsimd.dma_start(
                        tnr[0:h, c * DM:(c + 1) * DM],
                        tnat[s, 128 * c:128 * c + h, :])
                tn.append(tnr)
            p45cm = ExitStack()
            pch = p45cm.enter_context(
                tc.tile_pool(name="pch", bufs=1, side="right"))
            pp5 = p45cm.enter_context(
                tc.tile_pool(name="pp5", bufs=1, space="PSUM"))
            p5s = p45cm.enter_context(
                tc.tile_pool(name="p5s", bufs=3, side="right"))
            vgt = pch.tile([128, NBLK], I32, tag="vg")
            hgt = pch.tile([128, NBLK], I32, tag="hg")
            with tc.tile_pool(name="pstg", bufs=STG_BUFS, side="right") as pstg:
                # chase chunks: rows high->low, GCH rows each, within one c-chunk
                chunks = []
                for k in range(NCHUNK - 1, -1, -1):
                    i0 = k * GCH
                    chunks.append((i0, i0 // 128))
                stg = {s: [] for s in range(SPC)}
                engs = {}
                jregs = {}
                vregs = {}
                for s in range(SPC):
                    eng = getattr(nc, CH_ENG[s])
                    engs[s] = eng
                    jregs[s] = eng.alloc_register(f"j{s}")
                    vregs[s] = eng.alloc_register(f"v{s}")
                    eng.reg_mov(jregs[s], T2 - 1)

                def stage(s, kidx):
                    i0, c = chunks[kidx]
                    p0 = i0 % 128
                    t_ = pstg.tile([1, GCH * T2], I32, tag=f"stg{s}",
                                   name=f"stg{s}")
                    engs[s].dma_start(
                        t_[0:1, :],
                        PACKi[p0:p0 + GCH, blk(c, s):blk(c, s) + T2])
                    return t_

                for d in range(STG_BUFS - 1):
                    for s in range(SPC):
                        stg[s].append(stage(s, d))
                for kidx in range(NCHUNK):
                    for s in range(SPC):
                        eng = engs[s]
                        if kidx + STG_BUFS - 1 < NCHUNK:
                            stg[s].append(stage(s, kidx + STG_BUFS - 1))
                        t_ = stg[s][kidx]
                        i0, c = chunks[kidx]
                        for r in range(GCH - 1, -1, -1):
                            i = i0 + r
                            jv = nc.s_assert_within(
                                eng.snap(jregs[s]), 0, T2 - 1,
                                skip_runtime_assert=True)
                            eng.reg_load(vregs[s],
                                         t_[0:1, bass.ds(jv + r * T2, 1)])
                            p = i % 128
                            eng.reg_save(vout[s][c][0:1, p:p + 1], vregs[s])
                            if p == 0 and c > 0:
                                # duplicate into the chunk below's hi slot
                                eng.reg_save(vout[s][c - 1][0:1, 128:129],
                                             vregs[s])
                            eng.reg_alu(jregs[s], vregs[s], 1024, AOT.mod)
            if dbg is not None:
                for s in range(SPC):
                    for c in range(ICH):
                        h = _ich(c)
                        nc.sync.dma_start(
                            dbg[f"vout{s}"][:, 128 * c:128 * c + h],
                            vout[s][c][:, 0:h])

            # ------------- phase 4b: bulk W from lo/hi scalars ----------
            # scatter per-row scalars to [i%128, (c,s)] layout;
            # lo(i) = vout[i] >> 10, hi(i) = vout[i+1] & 1023.
            # c descending: W columns for chunk c become available as soon as
            # the chase passes that chunk, so phase 5 can overlap the chase.
            Wg = pWgp.tile([128, ICH * SPC * T2], BF16, tag="Wg")
            nc.vector.memset(Wg[:, :], 0.0)
            if True:
                vg = vgt
                hg = hgt
                lof = pch.tile([128, NBLK], F32, tag="lof")
                hif = pch.tile([128, NBLK], F32, tag="hif")
                # launder the DMA-loaded teacher through DVE so every
                # matmul input is DVE-produced (PE carries one sem wait);
                # runs during the chase (depends only on the tn DMAs)
                tnc = []
                for s in range(SPC):
                    tl = p5s.tile([128, ICH * DM], BF16, tag=f"tnc{s}",
                                  name=f"tnc{s}")
                    for c in range(ICH):
                        h = _ich(c)
                        nc.vector.tensor_copy(
                            out=tl[0:h, c * DM:(c + 1) * DM],
                            in_=tn[s][0:h, c * DM:(c + 1) * DM])
                    tnc.append(tl)
                for c in range(ICH - 1, -1, -1):
                    h = _ich(c)
                    for s in range(SPC):
                        col = c * SPC + s
                        nc.gpsimd.dma_start(vg[0:h, col:col + 1],
                                            vout[s][c][0:1, 0:h])
                        nc.gpsimd.dma_start(hg[0:h, col:col + 1],
                                            vout[s][c][0:1, 1:h + 1])
                        nc.vector.tensor_scalar(
                            out=vg[0:h, col:col + 1], in0=vg[0:h, col:col + 1],
                            scalar1=10, scalar2=0,
                            op0=AOT.arith_shift_right, op1=AOT.bitwise_or)
                        nc.vector.tensor_scalar(
                            out=hg[0:h, col:col + 1], in0=hg[0:h, col:col + 1],
                            scalar1=1023, scalar2=0,
                            op0=AOT.bitwise_and, op1=AOT.bitwise_or)
                        nc.vector.tensor_copy(out=lof[0:h, col:col + 1],
                                              in_=vg[0:h, col:col + 1])
                        nc.vector.tensor_copy(out=hif[0:h, col:col + 1],
                                              in_=hg[0:h, col:col + 1])
                        wv = Wg[0:h, col * T2:(col + 1) * T2]
                        nc.vector.tensor_scalar(
                            out=wv, in0=jrow[0:h, :],
                            scalar1=lof[0:h, col:col + 1], scalar2=0.0,
                            op0=AOT.is_ge, op1=AOT.add)
                        hv = pch.tile([128, T2], BF16, tag="hv", bufs=2)
                        nc.vector.tensor_scalar(
                            out=hv[0:h, :], in0=jrow[0:h, :],
                            scalar1=hif[0:h, col:col + 1], scalar2=0.0,
                            op0=AOT.is_le, op1=AOT.add)
                        nc.vector.tensor_tensor(
                            out=wv, in0=wv, in1=hv[0:h, :], op=AOT.mult)
                if dbg is not None:
                    with tc.tile_pool(name="dbgw", bufs=1) as dp_:
                        wtmp = dp_.tile([128, ICH * SPC * T2], F32, tag="wtmp")
                        nc.vector.tensor_copy(out=wtmp[:, :], in_=Wg[:, :])
                        nc.sync.dma_start(dbg["Wg"][:, :], wtmp[:, :])

                # -------- phase 5: out = W^T @ teacher (grouped PSUM) -------
                # 20 output tiles (s, jm, n2) in groups of 8 PSUM banks;
                # within a group accumulate c = 3,2,1,0 so the early matmuls
                # run while the chase is still producing low chunks.
                tiles = [(s, jm, n2) for s in range(SPC)
                         for jm in range(T2 // MT) for n2 in range(DM // 512)]
                pstiles = {}
                for gi in range(0, len(tiles), 8):
                    grp = tiles[gi:gi + 8]
                    for t in grp:
                        pstiles[t] = pp5.tile([MT, 512], F32,
                                              tag=f"ps{tiles.index(t) % 8}",
                                              name=f"ps{tiles.index(t) % 8}")
                    for c in range(ICH - 1, -1, -1):
                        h = _ich(c)
                        for t in grp:
                            s, jm, n2 = t
                            wbase = (c * SPC + s) * T2 + jm * MT
                            nc.tensor.matmul(
                                pstiles[t][:, :],
                                lhsT=Wg[0:h, wbase:wbase + MT],
                                rhs=tnc[s][0:h, c * DM + 512 * n2:
                                           c * DM + 512 * (n2 + 1)],
                                start=(c == ICH - 1), stop=(c == 0))
                    for t in grp:
                        s, jm, n2 = t
                        ob = p5s.tile([MT, 512], F32, tag="ob")
                        nc.vector.tensor_copy(out=ob[:, :], in_=pstiles[t][:, :])
                        nc.sync.dma_start(
                            out[s, jm * MT:(jm + 1) * MT,
                                512 * n2:512 * (n2 + 1)], ob[:, :])
            p45cm.close()
            esR.close()
            esD.close()
            esP.close()
    return nc


_CACHE = {}


def _get_nc():
    if "nc" not in _CACHE:
        nc = bacc.Bacc("TRN2", target_bir_lowering=False, debug=False)
        build_kernel(nc)
        nc.finalize()
        _CACHE["nc"] = nc
    return _CACHE["nc"]


def _to_bf16(a):
    import ml_dtypes
    return np.asarray(a, dtype=ml_dtypes.bfloat16)


def make_in_maps(t, s):
    in_maps = []
    for c in range(NCORES):
        tc_ = t[SPC * c:SPC * (c + 1)]
        sc_ = s[SPC * c:SPC * (c + 1)]
        in_maps.append({
            "tT": np.ascontiguousarray(tc_.transpose(0, 2, 1)),
            "sT": np.ascontiguousarray(sc_.transpose(0, 2, 1)),
            "tnat": _to_bf16(tc_),
        })
    return in_maps


def kernel(teacher_features: np.ndarray, student_features: np.ndarray) -> np.ndarray:
    from concourse.bass_utils import run_bass_kernel_spmd

    t = np.ascontiguousarray(np.asarray(teacher_features, dtype=np.float32))
    s = np.ascontiguousarray(np.asarray(student_features, dtype=np.float32))
    nc = _get_nc()
    res = run_bass_kernel_spmd(nc, make_in_maps(t, s), core_ids=list(range(NCORES)))
    return np.concatenate([res.results[c]["out"] for c in range(NCORES)], axis=0)
